# revision 28
# baseline (speedup 1.0000x reference)
"""LoRA layer kernel for Trainium2 (8 NeuronCores, data-parallel).

Computes out = SCALING * (x @ A^T) @ B^T for x [4, 8192, 1024],
lora_A [4, 1024], lora_B [1024, 4], SCALING = 0.25.

Strategy (per core, shard = 4096 rows x 1024 features), memory-bound:
  - Host pre-transposes and pre-rounds x to bf16 in the exact SBUF slab
    layout [slab][p][chunk][row]: every load is one DMA with 8 KiB
    per-partition contiguous lines and NO on-chip transpose. Output is
    written bf16 in a packed [slab][p][j][o] layout (host un-permutes
    and upcasts). Per-core HBM traffic: 8 MiB in + 8 MiB out.
  - mm1 (rank projection): A's 4 columns are replicated into PE array
    columns {0-3, 32-35, 64-67, 96-99} with zeros between (host-prepared
    weights), so the 8 chunk-accumulation matmuls produce h^T already
    replicated at 4 PSUM partition offsets - free replication for the
    row-tiled second stage, with exact zeros elsewhere.
  - mm2: 4 concurrent row-tiled matmuls (tile_position=(32r, 0)); tile r
    reads jtile r's h^T from partitions 32r..32r+3 and streams its own
    B half, so 4 jtiles finish in ~one N=512 stream time.
  - Pipeline: each slab's second mm2 wave is deferred past the next
    slab's mm1 chain so PSUM banks are evacuated under matmul cover
    (bank budget: htx 1 + per-r bufs (2,2,2,1) = 8; the single-buffered
    r=3 bank drains first on the engine opposite the ht evacuation).
  - Loads ride the sync (HWDGE) ring; slab 0 is split into per-chunk
    DMAs with separate tiles so the first matmul gates on 128 KiB, and
    the last slab is split into two 256-row mini-steps with a 2x256 KiB
    final store to halve the pipeline drain. Stores ride the gpsimd
    (SWDGE) ring so they never head-of-line-block loads.
"""

import sys

for _p in (
    "/root/.axon_site",
    "/root/.axon_site/_ro/trn_rl_repo",
    "/root/.axon_site/_ro/pypackages",
):
    if _p not in sys.path:
        sys.path.insert(0, _p)

from contextlib import ExitStack

import numpy as np
import ml_dtypes

BF16 = ml_dtypes.bfloat16

N_CORES = 8
D_IN = 1024
D_OUT = 1024
RANK = 4
ROWS_TOTAL = 4 * 8192
ROWS_PER_CORE = ROWS_TOTAL // N_CORES  # 4096
SCALING = 1.0 / RANK

P = 128            # partitions
CH = D_IN // P     # 8 feature chunks
SLAB = 512         # rows per pipeline step
NSLAB = ROWS_PER_CORE // SLAB  # 8
J = SLAB // P      # 4 row subtiles per slab (= row-tile lanes in mm2)
NO2 = D_OUT // 512  # 2 output column chunks of 512


def emit_lora(tc, xt_ap, at_ap, bt_ap, out_ap):
    """Emit the LoRA kernel IR for one core's shard.

    xt_ap : DRAM [NSLAB, P, CH, SLAB] bf16, xt[s, p, c, r] = x[s*SLAB+r, c*P+p]
    at_ap : DRAM [P, CH, P] bf16, at[p, c, 32g+r] = A[r, c*P+p] (g<4, r<4), 0 else
    bt_ap : DRAM [P, D_OUT] bf16, bt[32g+r, o] = SCALING * B[o, r] (g<4), 0 else
    out_ap: DRAM [NSLAB, P, J, D_OUT] bf16, out[s, p, j, o] = y[s*SLAB+j*P+p, o]
    """
    import concourse.mybir as mybir

    nc = tc.nc
    f32 = mybir.dt.float32
    bf16 = mybir.dt.bfloat16
    ctx = tc._ctx  # ExitStack owned by caller

    consts = ctx.enter_context(tc.tile_pool(name="consts", bufs=1))
    xpool = ctx.enter_context(tc.tile_pool(name="xt", bufs=7))
    htpool = ctx.enter_context(tc.tile_pool(name="ht", bufs=4))
    opool = ctx.enter_context(tc.tile_pool(name="osb", bufs=4))
    # 8 PSUM banks total: htx 1 + o_r bufs (2,2,2,1) = 8. Only r=3 is
    # single-buffered; its evacuation always goes first on the engine
    # opposite the ht evacuation so the next wave is never held up long.
    ps = ctx.enter_context(tc.tile_pool(name="ps", bufs=1, space="PSUM"))
    OR_BUFS = (2, 2, 2, 1)

    # The tiny constants lead the SWDGE ring; slab 0's load is split into
    # one DMA per chunk with SEPARATE tiles, so the first mm1 matmul gates
    # on 128 KiB (chunk 0) instead of the whole 1 MiB slab.
    at_sb = consts.tile([P, CH, P], bf16)
    nc.gpsimd.dma_start(at_sb[:], at_ap[:])
    bt_sb = consts.tile([P, D_OUT], bf16)
    nc.gpsimd.dma_start(bt_sb[:], bt_ap[:])

    xc0 = []
    for c in range(CH):
        xc = consts.tile([P, SLAB], bf16, name=f"xc{c}", tag=f"xc{c}")
        nc.sync.dma_start(xc[:], xt_ap[0, :, c, :])
        xc0.append(xc)

    def emit_mm1_half(htX_ps, xt_sb, half):
        # htX[32g+r, m] += sum_f A[r, cP+f] * x^T[cP+f, m] for each replica
        # g; zero weight columns leave exact zeros between the replicas.
        # The two half-chains of a slab bracket the previous slab's mm2
        # waves, which write other PSUM banks (group check is skipped).
        for c in range(4 * half, 4 * half + 4):
            rhs = xt_sb[c][:] if isinstance(xt_sb, list) else xt_sb[:, c, :]
            nc.tensor.matmul(
                htX_ps[:],
                lhsT=at_sb[:, c, :],
                rhs=rhs,
                start=(c == 0),
                stop=(c == CH - 1),
                skip_group_check=True,
            )

    def emit_wave(htX_sb, o_sb, o2, ht_on_vector, nj=J):
        o_ps = [
            ps.tile([P, 512], f32, name="o_ps", tag=f"o_r{r}", bufs=OR_BUFS[r])
            for r in range(nj)
        ]
        for r in range(nj):
            # out[m, o] = sum_r h^T[r, rP+m] * bt[r, o]; row-tile r of the
            # PE handles jtile r concurrently with the others.
            nc.tensor.matmul(
                o_ps[r][:],
                lhsT=htX_sb[32 * r : 32 * r + RANK, r * P : (r + 1) * P],
                rhs=bt_sb[32 * r : 32 * r + RANK, o2 * 512 : (o2 + 1) * 512],
                start=True,
                stop=True,
                tile_position=(32 * r, 0),
            )
        # r=3 (single-buffered bank) drains first on the engine opposite
        # the slab's ht evacuation; the rest split across both engines.
        order = ((3, not ht_on_vector), (2, ht_on_vector),
                 (1, not ht_on_vector), (0, ht_on_vector))
        for r, on_vector in order:
            if r >= nj:
                continue
            dst = o_sb[:, r, o2 * 512 : (o2 + 1) * 512]
            if on_vector:
                nc.vector.tensor_copy(dst, o_ps[r][:])
            else:
                nc.scalar.copy(dst, o_ps[r][:])

    # Pipeline steps: full 512-row slabs, with the LAST slab split into two
    # 256-row minis so the end-of-kernel drain chain is half as deep.
    steps = [(s, 0, J) for s in range(NSLAB - 1)]
    steps += [(NSLAB - 1, 0, J // 2), (NSLAB - 1, J // 2, J // 2)]

    pending = None  # (htX_sb, o_sb, s, j0, nj, htv) awaiting wave o2=1
    xt_last = None
    for k, (s, j0, nj) in enumerate(steps):
        if s == 0:
            xt_sb = xc0
        elif j0 == 0:
            xt_sb = xpool.tile([P, CH, SLAB], bf16)
            nc.sync.dma_start(xt_sb[:], xt_ap[s])
            xt_last = xt_sb
        else:
            xt_sb = xt_last

        rows = slice(j0 * P, (j0 + nj) * P)
        htX_ps = ps.tile([P, nj * P], f32, name="htX_ps", tag="htx", bufs=1)
        for half in range(2):
            for c in range(4 * half, 4 * half + 4):
                rhs = (
                    xt_sb[c][:, rows]
                    if isinstance(xt_sb, list)
                    else xt_sb[:, c, rows]
                )
                nc.tensor.matmul(
                    htX_ps[:],
                    lhsT=at_sb[:, c, :],
                    rhs=rhs,
                    start=(c == 0),
                    stop=(c == CH - 1),
                    skip_group_check=True,
                )

        ht_on_vector = k % 2 == 0
        htX_sb = htpool.tile([P, nj * P], bf16, name="htX_sb", tag="htx_sb")
        if ht_on_vector:
            nc.vector.tensor_copy(htX_sb[:], htX_ps[:])
        else:
            nc.scalar.copy(htX_sb[:], htX_ps[:])

        if pending is not None:
            p_ht, p_osb, p_s, p_j0, p_nj, p_htv = pending
            emit_wave(p_ht, p_osb, 1, p_htv, p_nj)
            # Stores ride the SWDGE (gpsimd) ring so a store waiting on its
            # copy never head-of-line-blocks the HWDGE load ring.
            nc.gpsimd.dma_start(
                out_ap[p_s, :, p_j0 : p_j0 + p_nj, :], p_osb[:]
            )

        o_sb = opool.tile([P, nj, D_OUT], bf16, name="o_sb", tag="o_sb")
        emit_wave(htX_sb, o_sb, 0, ht_on_vector, nj)
        pending = (htX_sb, o_sb, s, j0, nj, ht_on_vector)

    # Tail: the last mini-step's second wave; split its store per o2 so the
    # final DMA is 256 KiB fired right after the second wave's evacs.
    p_ht, p_osb, p_s, p_j0, p_nj, p_htv = pending
    dst = out_ap[p_s, :, p_j0 : p_j0 + p_nj, :].rearrange(
        "p j (o2 o) -> p j o2 o", o2=2
    )
    nc.sync.dma_start(dst[:, :, 0, :], p_osb[:, :, 0:512])
    emit_wave(p_ht, p_osb, 1, p_htv, p_nj)
    nc.sync.dma_start(dst[:, :, 1, :], p_osb[:, :, 512:1024])


def build_nc():
    import concourse.mybir as mybir
    import concourse.tile as tile
    from concourse import bacc

    bf16 = mybir.dt.bfloat16
    nc = bacc.Bacc("TRN2", target_bir_lowering=False, debug=False)
    xt_d = nc.dram_tensor(
        "xt", [NSLAB, P, CH, SLAB], bf16, kind="ExternalInput"
    ).ap()
    at_d = nc.dram_tensor("at", [P, CH, P], bf16, kind="ExternalInput").ap()
    bt_d = nc.dram_tensor("bt", [P, D_OUT], bf16, kind="ExternalInput").ap()
    out_d = nc.dram_tensor(
        "out", [NSLAB, P, J, D_OUT], bf16, kind="ExternalOutput"
    ).ap()

    with tile.TileContext(nc) as tc:
        with ExitStack() as ctx:
            tc._ctx = ctx
            emit_lora(tc, xt_d, at_d, bt_d, out_d)
    nc.compile()
    return nc


def host_prep_x(x2):
    """f32 [ROWS_TOTAL, D_IN] -> per-core bf16 [NSLAB, P, CH, SLAB]."""
    xb = x2.astype(BF16)
    shards = xb.reshape(N_CORES, NSLAB, SLAB, CH, P)
    return [
        np.ascontiguousarray(shards[i].transpose(0, 3, 2, 1))
        for i in range(N_CORES)
    ]


def host_prep_ab(lora_A, lora_B):
    # at[p, c, 32g+r] = A[r, c*P+p] for g in 0..3, zeros elsewhere
    a_pcr = (
        np.asarray(lora_A, dtype=np.float32)
        .T.reshape(CH, P, RANK)
        .transpose(1, 0, 2)
    )  # [P, CH, RANK]
    at = np.zeros((P, CH, P), dtype=np.float32)
    for g in range(4):
        at[:, :, 32 * g : 32 * g + RANK] = a_pcr
    # bt[32g+r, o] = SCALING * B[o, r], zeros elsewhere
    b_ro = np.asarray(lora_B, dtype=np.float32).T * SCALING  # [RANK, D_OUT]
    bt = np.zeros((P, D_OUT), dtype=np.float32)
    for g in range(4):
        bt[32 * g : 32 * g + RANK, :] = b_ro
    return np.ascontiguousarray(at.astype(BF16)), np.ascontiguousarray(
        bt.astype(BF16)
    )


def host_unpack_out(bufs):
    """Per-core bf16 [NSLAB, P, J, D_OUT] -> f32 [4, 8192, D_OUT]."""
    full = np.stack([np.asarray(b) for b in bufs], axis=0)
    # row = s*SLAB + j*P + p
    full = full.transpose(0, 1, 3, 2, 4).reshape(ROWS_TOTAL, D_OUT)
    return full.astype(np.float32).reshape(4, 8192, D_OUT)


_NC_CACHE = {}


def kernel(x, lora_A, lora_B):
    from concourse.bass_utils import run_bass_kernel_spmd

    if "nc" not in _NC_CACHE:
        _NC_CACHE["nc"] = build_nc()
    nc = _NC_CACHE["nc"]

    x2 = np.ascontiguousarray(x, dtype=np.float32).reshape(ROWS_TOTAL, D_IN)
    xts = host_prep_x(x2)
    at, bt = host_prep_ab(lora_A, lora_B)
    in_maps = [{"xt": xts[i], "at": at, "bt": bt} for i in range(N_CORES)]
    res = run_bass_kernel_spmd(nc, in_maps, core_ids=list(range(N_CORES)))
    return host_unpack_out([res.results[i]["out"] for i in range(N_CORES)])


# revision 30
# speedup vs baseline: 1.0487x; 1.0487x over previous
"""LoRA layer kernel for Trainium2 (8 NeuronCores, data-parallel).

Computes out = SCALING * (x @ A^T) @ B^T for x [4, 8192, 1024],
lora_A [4, 1024], lora_B [1024, 4], SCALING = 0.25.

Strategy (per core, shard = 4096 rows x 1024 features), memory-bound:
  - Host pre-transposes and pre-rounds x to bf16 in the exact SBUF slab
    layout [slab][p][chunk][row]: every load is one DMA with 8 KiB
    per-partition contiguous lines and NO on-chip transpose. Output is
    written bf16 in a packed [slab][p][j][o] layout (host un-permutes
    and upcasts). Per-core HBM traffic: 8 MiB in + 8 MiB out.
  - mm1 (rank projection): A's 4 columns are replicated into PE array
    columns {0-3, 32-35, 64-67, 96-99} with zeros between (host-prepared
    weights), so the 8 chunk-accumulation matmuls produce h^T already
    replicated at 4 PSUM partition offsets - free replication for the
    row-tiled second stage, with exact zeros elsewhere.
  - mm2: 4 concurrent row-tiled matmuls (tile_position=(32r, 0)); tile r
    reads jtile r's h^T from partitions 32r..32r+3 and streams its own
    B half, so 4 jtiles finish in ~one N=512 stream time.
  - Pipeline: each slab's second mm2 wave is deferred past the next
    slab's mm1 chain so PSUM banks are evacuated under matmul cover
    (bank budget: htx 1 + per-r bufs (2,2,2,1) = 8; the single-buffered
    r=3 bank drains first on the engine opposite the ht evacuation).
  - Loads ride the sync (HWDGE) ring; slab 0 is split into per-chunk
    DMAs with separate tiles so the first matmul gates on 128 KiB, and
    the last slab is split into two 256-row mini-steps with a 2x256 KiB
    final store to halve the pipeline drain. Stores ride the gpsimd
    (SWDGE) ring so they never head-of-line-block loads.
"""

import sys

for _p in (
    "/root/.axon_site",
    "/root/.axon_site/_ro/trn_rl_repo",
    "/root/.axon_site/_ro/pypackages",
):
    if _p not in sys.path:
        sys.path.insert(0, _p)

from contextlib import ExitStack

import numpy as np
import ml_dtypes

BF16 = ml_dtypes.bfloat16

N_CORES = 8
D_IN = 1024
D_OUT = 1024
RANK = 4
ROWS_TOTAL = 4 * 8192
ROWS_PER_CORE = ROWS_TOTAL // N_CORES  # 4096
SCALING = 1.0 / RANK

P = 128            # partitions
CH = D_IN // P     # 8 feature chunks
SLAB = 512         # rows per pipeline step
NSLAB = ROWS_PER_CORE // SLAB  # 8
J = SLAB // P      # 4 row subtiles per slab (= row-tile lanes in mm2)
NO2 = D_OUT // 512  # 2 output column chunks of 512


def emit_lora(tc, xt_ap, at_ap, bt_ap, out_ap):
    """Emit the LoRA kernel IR for one core's shard.

    xt_ap : DRAM [NSLAB, P, CH, SLAB] bf16, xt[s, p, c, r] = x[s*SLAB+r, c*P+p]
    at_ap : DRAM [P, CH, P] bf16, at[p, c, 32g+r] = A[r, c*P+p] (g<4, r<4), 0 else
    bt_ap : DRAM [P, D_OUT] bf16, bt[32g+r, o] = SCALING * B[o, r] (g<4), 0 else
    out_ap: DRAM [NSLAB, P, J, D_OUT] bf16, out[s, p, j, o] = y[s*SLAB+j*P+p, o]
    """
    import concourse.mybir as mybir

    nc = tc.nc
    f32 = mybir.dt.float32
    bf16 = mybir.dt.bfloat16
    ctx = tc._ctx  # ExitStack owned by caller

    consts = ctx.enter_context(tc.tile_pool(name="consts", bufs=1))
    xpool = ctx.enter_context(tc.tile_pool(name="xt", bufs=7))
    htpool = ctx.enter_context(tc.tile_pool(name="ht", bufs=4))
    opool = ctx.enter_context(tc.tile_pool(name="osb", bufs=4))
    # 8 PSUM banks total: htx 1 + o_r bufs (2,2,2,1) = 8. Only r=3 is
    # single-buffered; its evacuation always goes first on the engine
    # opposite the ht evacuation so the next wave is never held up long.
    ps = ctx.enter_context(tc.tile_pool(name="ps", bufs=1, space="PSUM"))
    OR_BUFS = (2, 2, 2, 1)

    # HAM warm-up: the PE is idle from engine-init (~7.5us) until the
    # first data lands (~11.5us) - exactly one 3.4us HAM activity window.
    # A burst of dummy matmuls on memset data flips the clock gate to
    # K=8/8 so the pipeline-fill slabs run at 2.4 GHz instead of 1.2,
    # pulling the whole trailing store stream ~2us earlier.
    warm_sb = consts.tile([P, P], bf16, name="warm_sb", tag="warm_sb")
    nc.gpsimd.memset(warm_sb[:], 0.0)
    warm_ps = ps.tile([P, 512], f32, name="warm_ps", tag="htx", bufs=1)
    for _ in range(16):
        nc.tensor.matmul(
            warm_ps[:, 0:P], lhsT=warm_sb[:], rhs=warm_sb[:],
            start=True, stop=True,
        )

    # The tiny constants lead the SWDGE ring; slab 0's load is split into
    # one DMA per chunk with SEPARATE tiles, so the first mm1 matmul gates
    # on 128 KiB (chunk 0) instead of the whole 1 MiB slab.
    at_sb = consts.tile([P, CH, P], bf16)
    nc.gpsimd.dma_start(at_sb[:], at_ap[:])
    bt_sb = consts.tile([P, D_OUT], bf16)
    nc.gpsimd.dma_start(bt_sb[:], bt_ap[:])

    xc0 = []
    for c in range(CH):
        xc = consts.tile([P, SLAB], bf16, name=f"xc{c}", tag=f"xc{c}")
        nc.sync.dma_start(xc[:], xt_ap[0, :, c, :])
        xc0.append(xc)

    def emit_mm1_half(htX_ps, xt_sb, half):
        # htX[32g+r, m] += sum_f A[r, cP+f] * x^T[cP+f, m] for each replica
        # g; zero weight columns leave exact zeros between the replicas.
        # The two half-chains of a slab bracket the previous slab's mm2
        # waves, which write other PSUM banks (group check is skipped).
        for c in range(4 * half, 4 * half + 4):
            rhs = xt_sb[c][:] if isinstance(xt_sb, list) else xt_sb[:, c, :]
            nc.tensor.matmul(
                htX_ps[:],
                lhsT=at_sb[:, c, :],
                rhs=rhs,
                start=(c == 0),
                stop=(c == CH - 1),
                skip_group_check=True,
            )

    def emit_wave(htX_sb, o_sb, o2, ht_on_vector, nj=J):
        o_ps = [
            ps.tile([P, 512], f32, name="o_ps", tag=f"o_r{r}", bufs=OR_BUFS[r])
            for r in range(nj)
        ]
        for r in range(nj):
            # out[m, o] = sum_r h^T[r, rP+m] * bt[r, o]; row-tile r of the
            # PE handles jtile r concurrently with the others.
            nc.tensor.matmul(
                o_ps[r][:],
                lhsT=htX_sb[32 * r : 32 * r + RANK, r * P : (r + 1) * P],
                rhs=bt_sb[32 * r : 32 * r + RANK, o2 * 512 : (o2 + 1) * 512],
                start=True,
                stop=True,
                tile_position=(32 * r, 0),
            )
        # r=3 (single-buffered bank) drains first on the engine opposite
        # the slab's ht evacuation; the rest split across both engines.
        order = ((3, not ht_on_vector), (2, ht_on_vector),
                 (1, not ht_on_vector), (0, ht_on_vector))
        for r, on_vector in order:
            if r >= nj:
                continue
            dst = o_sb[:, r, o2 * 512 : (o2 + 1) * 512]
            if on_vector:
                nc.vector.tensor_copy(dst, o_ps[r][:])
            else:
                nc.scalar.copy(dst, o_ps[r][:])

    # Pipeline steps: full 512-row slabs, with the LAST slab split into two
    # 256-row minis so the end-of-kernel drain chain is half as deep.
    steps = [(s, 0, J) for s in range(NSLAB - 1)]
    steps += [(NSLAB - 1, 0, J // 2), (NSLAB - 1, J // 2, J // 2)]

    pending = None  # (htX_sb, o_sb, s, j0, nj, htv) awaiting wave o2=1
    xt_last = None
    for k, (s, j0, nj) in enumerate(steps):
        if s == 0:
            xt_sb = xc0
        elif j0 == 0:
            xt_sb = xpool.tile([P, CH, SLAB], bf16)
            # Slabs 1-2 load via the scalar (ACT) HWDGE ring: the sync
            # queue is busy ~6us issuing slab 0's eight chunk DMAs, which
            # otherwise delays slab 1's data to ~19.5us and stalls mm1(1)
            # by ~3.4us - exactly one HAM window, triggering a 10us
            # re-throttle. The scalar queue is empty this early (its evac
            # duties start ~15us), so there is no head-of-line conflict.
            eng = nc.scalar if s in (1, 2) else nc.sync
            eng.dma_start(xt_sb[:], xt_ap[s])
            xt_last = xt_sb
        else:
            xt_sb = xt_last

        rows = slice(j0 * P, (j0 + nj) * P)
        htX_ps = ps.tile([P, nj * P], f32, name="htX_ps", tag="htx", bufs=1)
        for half in range(2):
            for c in range(4 * half, 4 * half + 4):
                rhs = (
                    xt_sb[c][:, rows]
                    if isinstance(xt_sb, list)
                    else xt_sb[:, c, rows]
                )
                nc.tensor.matmul(
                    htX_ps[:],
                    lhsT=at_sb[:, c, :],
                    rhs=rhs,
                    start=(c == 0),
                    stop=(c == CH - 1),
                    skip_group_check=True,
                )

        ht_on_vector = k % 2 == 0
        htX_sb = htpool.tile([P, nj * P], bf16, name="htX_sb", tag="htx_sb")
        if ht_on_vector:
            nc.vector.tensor_copy(htX_sb[:], htX_ps[:])
        else:
            nc.scalar.copy(htX_sb[:], htX_ps[:])

        if pending is not None:
            p_ht, p_osb, p_s, p_j0, p_nj, p_htv = pending
            emit_wave(p_ht, p_osb, 1, p_htv, p_nj)
            # Stores ride the SWDGE (gpsimd) ring so a store waiting on its
            # copy never head-of-line-blocks the HWDGE load ring.
            nc.gpsimd.dma_start(
                out_ap[p_s, :, p_j0 : p_j0 + p_nj, :], p_osb[:]
            )

        o_sb = opool.tile([P, nj, D_OUT], bf16, name="o_sb", tag="o_sb")
        emit_wave(htX_sb, o_sb, 0, ht_on_vector, nj)
        pending = (htX_sb, o_sb, s, j0, nj, ht_on_vector)

    # Tail: the last mini-step's second wave; split its store per o2 so the
    # final DMA is 256 KiB fired right after the second wave's evacs.
    p_ht, p_osb, p_s, p_j0, p_nj, p_htv = pending
    dst = out_ap[p_s, :, p_j0 : p_j0 + p_nj, :].rearrange(
        "p j (o2 o) -> p j o2 o", o2=2
    )
    nc.sync.dma_start(dst[:, :, 0, :], p_osb[:, :, 0:512])
    emit_wave(p_ht, p_osb, 1, p_htv, p_nj)
    nc.sync.dma_start(dst[:, :, 1, :], p_osb[:, :, 512:1024])


def build_nc():
    import concourse.mybir as mybir
    import concourse.tile as tile
    from concourse import bacc

    bf16 = mybir.dt.bfloat16
    nc = bacc.Bacc("TRN2", target_bir_lowering=False, debug=False)
    xt_d = nc.dram_tensor(
        "xt", [NSLAB, P, CH, SLAB], bf16, kind="ExternalInput"
    ).ap()
    at_d = nc.dram_tensor("at", [P, CH, P], bf16, kind="ExternalInput").ap()
    bt_d = nc.dram_tensor("bt", [P, D_OUT], bf16, kind="ExternalInput").ap()
    out_d = nc.dram_tensor(
        "out", [NSLAB, P, J, D_OUT], bf16, kind="ExternalOutput"
    ).ap()

    with tile.TileContext(nc) as tc:
        with ExitStack() as ctx:
            tc._ctx = ctx
            emit_lora(tc, xt_d, at_d, bt_d, out_d)
    nc.compile()
    return nc


def host_prep_x(x2):
    """f32 [ROWS_TOTAL, D_IN] -> per-core bf16 [NSLAB, P, CH, SLAB]."""
    xb = x2.astype(BF16)
    shards = xb.reshape(N_CORES, NSLAB, SLAB, CH, P)
    return [
        np.ascontiguousarray(shards[i].transpose(0, 3, 2, 1))
        for i in range(N_CORES)
    ]


def host_prep_ab(lora_A, lora_B):
    # at[p, c, 32g+r] = A[r, c*P+p] for g in 0..3, zeros elsewhere
    a_pcr = (
        np.asarray(lora_A, dtype=np.float32)
        .T.reshape(CH, P, RANK)
        .transpose(1, 0, 2)
    )  # [P, CH, RANK]
    at = np.zeros((P, CH, P), dtype=np.float32)
    for g in range(4):
        at[:, :, 32 * g : 32 * g + RANK] = a_pcr
    # bt[32g+r, o] = SCALING * B[o, r], zeros elsewhere
    b_ro = np.asarray(lora_B, dtype=np.float32).T * SCALING  # [RANK, D_OUT]
    bt = np.zeros((P, D_OUT), dtype=np.float32)
    for g in range(4):
        bt[32 * g : 32 * g + RANK, :] = b_ro
    return np.ascontiguousarray(at.astype(BF16)), np.ascontiguousarray(
        bt.astype(BF16)
    )


def host_unpack_out(bufs):
    """Per-core bf16 [NSLAB, P, J, D_OUT] -> f32 [4, 8192, D_OUT]."""
    full = np.stack([np.asarray(b) for b in bufs], axis=0)
    # row = s*SLAB + j*P + p
    full = full.transpose(0, 1, 3, 2, 4).reshape(ROWS_TOTAL, D_OUT)
    return full.astype(np.float32).reshape(4, 8192, D_OUT)


_NC_CACHE = {}


def kernel(x, lora_A, lora_B):
    from concourse.bass_utils import run_bass_kernel_spmd

    if "nc" not in _NC_CACHE:
        _NC_CACHE["nc"] = build_nc()
    nc = _NC_CACHE["nc"]

    x2 = np.ascontiguousarray(x, dtype=np.float32).reshape(ROWS_TOTAL, D_IN)
    xts = host_prep_x(x2)
    at, bt = host_prep_ab(lora_A, lora_B)
    in_maps = [{"xt": xts[i], "at": at, "bt": bt} for i in range(N_CORES)]
    res = run_bass_kernel_spmd(nc, in_maps, core_ids=list(range(N_CORES)))
    return host_unpack_out([res.results[i]["out"] for i in range(N_CORES)])
